# revision 1
# baseline (speedup 1.0000x reference)
"""Trainium2 Bass kernel for GatedCrossAttention (B=4, N=4096, C=1024, H=16, M=4).

Reference math (dead code removed: the v/gate projections are overwritten
by views of k in the original module, so v = g = k):
    q = query @ Wq.T + bq                    [B,N,C]   -> [B,N,H,hd]
    k = key   @ Wk.T + bk                    [B,N,M,C] -> [B,N,M,H,hd]
    attn = softmax_M(SCALE * einsum('bnhc,bnmhc->bnmh', q, k))
    out  = einsum('bnmh,bnmhc->bnhc', attn, k*k) . reshape(B,N,C)
    out  = out @ Wo.T + bo

Strategy: pure data parallel over the 16384 tokens (8 cores x 2048), no
collectives.  On-chip layout is "transposed": channels on partitions, tokens
on the free axis, so every matmul contraction (over channels) is a natural
PE op.  The per-head segment reductions use indicator matmuls with the
elementwise product q*k as the stationary operand, landing logits token-major
([t, (m,h)]) so the M-softmax runs on all 128 partitions; a tiny PE transpose
of the softmax weights returns them head-major for the head->channel
broadcast matmuls.  Host pre-transposes/casts inputs and weights to fp16
(error vs f32 reference ~1e-3, PE runs 16-bit at full rate), accumulation
stays f32 in PSUM.
"""

import dataclasses
import numpy as np
from contextlib import ExitStack

try:
    import concourse.bass as bass
except ImportError:  # path fallback for bare containers
    import sys

    sys.path.insert(0, "/opt/trn_rl_repo")
    import concourse.bass as bass

import concourse.tile as tile
from concourse import bacc, mybir
from concourse.bass_utils import run_bass_kernel_spmd
from concourse.masks import make_identity

# problem constants (hardcoded per the task contract)
B, N, C, H, HD, M = 4, 4096, 1024, 16, 64, 4
SCALE = float(HD) ** -0.5
NCORES = 8
T_TOTAL = B * N
T_CORE = T_TOTAL // NCORES  # 2048
TB = 512                    # tokens per block (one PSUM bank at f32)
NJ = C // 128               # 8 channel tiles
NT = TB // 128              # 4 token subtiles per block

DT = mybir.dt.float16
NPDT = np.float16
F32 = mybir.dt.float32


def _bcast(ap, reps, axis):
    """Insert a 0-stride dim of size `reps` at AP position `axis` (0=partition)."""
    new = list(ap.ap)
    new.insert(axis, [0, reps])
    return dataclasses.replace(ap, ap=new)


def build_nc(t_core=T_CORE, with_bias=False):
    nblk = t_core // TB
    nc = bacc.Bacc("TRN2", target_bir_lowering=False, debug=False)

    qT = nc.declare_dram_parameter("qT", [C, t_core], DT, isOutput=False)
    kT = nc.declare_dram_parameter("kT", [M, C, t_core], DT, isOutput=False)
    wqT = nc.declare_dram_parameter("wqT", [C, C], DT, isOutput=False)
    wkT = nc.declare_dram_parameter("wkT", [C, C], DT, isOutput=False)
    woT = nc.declare_dram_parameter("woT", [C, C], DT, isOutput=False)
    indl = nc.declare_dram_parameter("indl", [128, 2], DT, isOutput=False)
    indb = nc.declare_dram_parameter("indb", [M, NJ, 64, 128], DT, isOutput=False)
    if with_bias:
        bq = nc.declare_dram_parameter("bq", [1, C], DT, isOutput=False)
        bk = nc.declare_dram_parameter("bk", [1, C], DT, isOutput=False)
        bo = nc.declare_dram_parameter("bo", [1, C], DT, isOutput=False)
    out = nc.declare_dram_parameter("out", [t_core, C], F32, isOutput=True)

    # DRAM views: channel dim split into (chunk, partition)
    qT_v = qT.ap().rearrange("(c p) t -> p c t", p=128)
    kT_v = kT.ap().rearrange("m (c p) t -> p m c t", p=128)
    wq_v = wqT.ap().rearrange("(c p) j -> p c j", p=128)
    wk_v = wkT.ap().rearrange("(c p) j -> p c j", p=128)
    wo_v = woT.ap().rearrange("(c p) j -> p c j", p=128)

    with tile.TileContext(nc) as tc, ExitStack() as ctx:
        consts = ctx.enter_context(tc.tile_pool(name="consts", bufs=1))
        p_inq = ctx.enter_context(tc.tile_pool(name="inq", bufs=2))
        p_ink = ctx.enter_context(tc.tile_pool(name="ink", bufs=2))
        p_qp = ctx.enter_context(tc.tile_pool(name="qp", bufs=9))
        p_kp = ctx.enter_context(tc.tile_pool(name="kp", bufs=8))
        p_prod = ctx.enter_context(tc.tile_pool(name="prod", bufs=2))
        p_ksq = ctx.enter_context(tc.tile_pool(name="ksq", bufs=2))
        p_sm = ctx.enter_context(tc.tile_pool(name="sm", bufs=2))
        p_ct = ctx.enter_context(tc.tile_pool(name="ct", bufs=2))
        p_y = ctx.enter_context(tc.tile_pool(name="y", bufs=2))
        p_yb = ctx.enter_context(tc.tile_pool(name="yb", bufs=10))
        p_out = ctx.enter_context(tc.tile_pool(name="outs", bufs=3))
        pp = ctx.enter_context(tc.tile_pool(name="pp", bufs=2, space="PSUM"))
        pl = ctx.enter_context(tc.tile_pool(name="pl", bufs=2, space="PSUM"))
        pw = ctx.enter_context(tc.tile_pool(name="pw", bufs=2, space="PSUM"))
        pb = ctx.enter_context(tc.tile_pool(name="pb", bufs=2, space="PSUM"))

        # ---- constants / weights (resident) ----
        wq_sb = consts.tile([128, NJ, C], DT)
        wk_sb = consts.tile([128, NJ, C], DT)
        wo_sb = consts.tile([128, NJ, C], DT)
        nc.sync.dma_start(out=wq_sb, in_=wq_v)
        nc.sync.dma_start(out=wk_sb, in_=wk_v)
        nc.sync.dma_start(out=wo_sb, in_=wo_v)
        indl_sb = consts.tile([128, 2], DT)
        indb_sb = consts.tile([64, M, NJ, 128], DT)
        nc.sync.dma_start(out=indl_sb, in_=indl.ap())
        nc.sync.dma_start(out=indb_sb, in_=indb.ap().rearrange("m r p j -> p m r j"))
        ident = consts.tile([128, 128], DT)
        make_identity(nc, ident)
        if with_bias:
            ones_sb = consts.tile([1, TB], DT)
            nc.vector.memset(ones_sb, 1.0)
            bq_sb = consts.tile([1, C], DT)
            bk_sb = consts.tile([1, C], DT)
            bo_sb = consts.tile([1, C], DT)
            nc.sync.dma_start(out=bq_sb, in_=bq.ap())
            nc.sync.dma_start(out=bk_sb, in_=bk.ap())
            nc.sync.dma_start(out=bo_sb, in_=bo.ap())

        for blk in range(nblk):
            t0 = blk * TB
            tsl = slice(t0, t0 + TB)

            # ---- load inputs ----
            q_in = p_inq.tile([128, NJ, TB], DT)
            nc.sync.dma_start(out=q_in, in_=qT_v[:, :, tsl])
            k_in = [p_ink.tile([128, NJ, TB], DT, tag="kin", name="kin")
                    for _ in range(M)]
            for m in range(M):
                nc.sync.dma_start(out=k_in[m], in_=kT_v[:, m, :, tsl])

            # ---- projections (PE) ----
            qp = [p_qp.tile([128, TB], DT, tag="qp", name="qp") for _ in range(NJ)]
            for r in range(NJ):
                ps = pp.tile([128, TB], F32, tag="pp", name="pp")
                for c in range(NJ):
                    nc.tensor.matmul(
                        ps,
                        wq_sb[:, c, r * 128:(r + 1) * 128],
                        q_in[:, c, :],
                        start=(c == 0),
                        stop=(c == NJ - 1 and not with_bias),
                    )
                if with_bias:
                    nc.tensor.matmul(
                        ps, bq_sb[:, r * 128:(r + 1) * 128], ones_sb,
                        start=False, stop=True,
                    )
                nc.scalar.copy(out=qp[r], in_=ps)

            kp = [p_kp.tile([128, M, TB], DT, tag="kp", name="kp")
                  for _ in range(NJ)]
            for m in range(M):
                for r in range(NJ):
                    ps = pp.tile([128, TB], F32, tag="pp", name="pp")
                    for c in range(NJ):
                        nc.tensor.matmul(
                            ps,
                            wk_sb[:, c, r * 128:(r + 1) * 128],
                            k_in[m][:, c, :],
                            start=(c == 0),
                            stop=(c == NJ - 1 and not with_bias),
                        )
                    if with_bias:
                        nc.tensor.matmul(
                            ps, bk_sb[:, r * 128:(r + 1) * 128], ones_sb,
                            start=False, stop=True,
                        )
                    nc.scalar.copy(out=kp[r][:, m, :], in_=ps)

            # ---- attention logits, token-major: pslt[t, tt, m, h] ----
            pslt = pl.tile([128, NT, M, H], F32, tag="pl", name="pl")
            for r in range(NJ):
                prod = p_prod.tile([128, M, TB], DT, tag="prod", name="prod")
                nc.vector.tensor_mul(prod, _bcast(qp[r], M, 1), kp[r])
                for tt in range(NT):
                    for m in range(M):
                        nc.tensor.matmul(
                            pslt[:, tt, m, 2 * r:2 * r + 2],
                            prod[:, m, tt * 128:(tt + 1) * 128],
                            indl_sb,
                            start=True,
                            stop=True,
                        )

            # ---- softmax over M (token-major, full 128 partitions) ----
            e = p_sm.tile([128, NT, M, H], F32, tag="e", name="e")
            nc.scalar.activation(e, pslt, func=mybir.ActivationFunctionType.Exp)
            s01 = p_sm.tile([128, NT, H], F32, tag="s01", name="s01")
            s = p_sm.tile([128, NT, H], F32, tag="s", name="s")
            nc.vector.tensor_add(s01, e[:, :, 0, :], e[:, :, 1, :])
            nc.vector.tensor_add(s, e[:, :, 2, :], e[:, :, 3, :])
            nc.vector.tensor_add(s, s01, s)
            rcp = p_sm.tile([128, NT, H], F32, tag="rcp", name="rcp")
            nc.vector.reciprocal(rcp, s)
            w_t = p_sm.tile([128, NT, M, H], DT, tag="w", name="w")
            nc.vector.tensor_mul(w_t, e, _bcast(rcp, M, 2))

            # transpose w to head-major: wT[(m,h), (tt,t)]
            wT = p_sm.tile([64, NT, 128], DT, tag="wT", name="wT")
            for tt in range(NT):
                pst = pw.tile([64, 128], DT, tag="pw", name="pw")
                nc.tensor.transpose(pst, w_t[:, tt, :, :], ident)
                nc.scalar.copy(out=wT[:, tt, :], in_=pst)

            # ---- weighted sum of k^2 (PE broadcast + DVE) ----
            yb = [p_yb.tile([128, TB], DT, tag="yb", name="yb") for _ in range(NJ)]
            for r in range(NJ):
                ksq = p_ksq.tile([128, M, TB], DT, tag="ksq", name="ksq")
                nc.vector.tensor_mul(ksq, kp[r], kp[r])
                ct = p_ct.tile([128, M, TB], F32, tag="ct", name="ct")
                for mp in range(2):      # m-pairs
                    for hf in range(2):  # half-blocks of 256 tokens
                        psb = pb.tile([128, 2, 256], F32, tag="pb", name="pb")
                        for mi in range(2):
                            m = 2 * mp + mi
                            for ti in range(2):
                                tt = 2 * hf + ti
                                nc.tensor.matmul(
                                    psb[:, mi, ti * 128:(ti + 1) * 128],
                                    indb_sb[:, m, r, :],
                                    wT[:, tt, :],
                                    start=True,
                                    stop=True,
                                )
                        nc.vector.tensor_mul(
                            ct[:, 2 * mp:2 * mp + 2, hf * 256:(hf + 1) * 256],
                            psb,
                            ksq[:, 2 * mp:2 * mp + 2, hf * 256:(hf + 1) * 256],
                        )
                y = p_y.tile([128, TB], F32, tag="y", name="y")
                nc.vector.reduce_sum(
                    y, ct.rearrange("p m t -> p t m"), axis=mybir.AxisListType.X
                )
                nc.scalar.copy(out=yb[r], in_=y)  # cast f32 -> fp16

            # ---- output projection (PE) ----
            for tt in range(NT):
                for oc in range(2):
                    ps = pp.tile([128, 512], F32, tag="pp", name="pp")
                    for r in range(NJ):
                        nc.tensor.matmul(
                            ps,
                            yb[r][:, tt * 128:(tt + 1) * 128],
                            wo_sb[:, r, oc * 512:(oc + 1) * 512],
                            start=(r == 0),
                            stop=(r == NJ - 1 and not with_bias),
                        )
                    if with_bias:
                        nc.tensor.matmul(
                            ps,
                            ones_sb[:, :128],
                            bo_sb[:, oc * 512:(oc + 1) * 512],
                            start=False,
                            stop=True,
                        )
                    o_sb = p_out.tile([128, 512], F32, tag="outs", name="osb")
                    nc.scalar.copy(out=o_sb, in_=ps)
                    nc.sync.dma_start(
                        out=out.ap()[t0 + tt * 128:t0 + (tt + 1) * 128,
                                     oc * 512:(oc + 1) * 512],
                        in_=o_sb,
                    )
    nc.compile()
    return nc


def _host_prep(query, key, Wq, Wk, Wo, bq, bk, bo):
    qT = np.ascontiguousarray(query.reshape(T_TOTAL, C).T).astype(NPDT)
    kT = np.ascontiguousarray(key.reshape(T_TOTAL, M, C).transpose(1, 2, 0)).astype(NPDT)

    wqT = np.ascontiguousarray(Wq.T).astype(NPDT)
    wkT = np.ascontiguousarray(Wk.T).astype(NPDT)
    woT = np.ascontiguousarray(Wo.T).astype(NPDT)

    j = np.arange(128)
    indl = np.stack([(j < 64), (j >= 64)], axis=1).astype(NPDT) * NPDT(SCALE)
    # indb[m, r, row, j] = 1 iff row == m*H + 2r + (j >= 64)
    rows = np.arange(64)[None, None, :, None]
    ms = np.arange(M)[:, None, None, None]
    rs = np.arange(NJ)[None, :, None, None]
    indb = (rows == ms * H + 2 * rs + (j[None, None, None, :] >= 64)).astype(NPDT)

    with_bias = bool(np.any(bq) or np.any(bk) or np.any(bo))
    common = {"wqT": wqT, "wkT": wkT, "woT": woT, "indl": indl, "indb": indb}
    if with_bias:
        common |= {
            "bq": bq.reshape(1, C).astype(NPDT),
            "bk": bk.reshape(1, C).astype(NPDT),
            "bo": bo.reshape(1, C).astype(NPDT),
        }
    in_maps = []
    for i in range(NCORES):
        sl = slice(i * T_CORE, (i + 1) * T_CORE)
        in_maps.append(
            {
                "qT": np.ascontiguousarray(qT[:, sl]),
                "kT": np.ascontiguousarray(kT[:, :, sl]),
                **common,
            }
        )
    return in_maps, with_bias


_NC_CACHE = {}
_LAST_RESULT = None


def kernel(query, key, gate, Wq, bq, Wk, bk, Wv, bv, Wg, bg, Wo, bo):
    in_maps, with_bias = _host_prep(query, key, Wq, Wk, Wo, bq, bk, bo)
    key_ = (T_CORE, with_bias)
    if key_ not in _NC_CACHE:
        _NC_CACHE[key_] = build_nc(T_CORE, with_bias)
    nc = _NC_CACHE[key_]
    res = run_bass_kernel_spmd(nc, in_maps, list(range(NCORES)))
    global _LAST_RESULT
    _LAST_RESULT = res
    out = np.concatenate([res.results[i]["out"] for i in range(NCORES)], axis=0)
    return out.reshape(B, N, C)



# revision 8
# speedup vs baseline: 1.6631x; 1.6631x over previous
"""Trainium2 Bass kernel for GatedCrossAttention (B=4, N=4096, C=1024, H=16, M=4).

Reference math (dead code removed: the v/gate projections are overwritten
by views of k in the original module, so v = g = k):
    q = query @ Wq.T + bq                    [B,N,C]   -> [B,N,H,hd]
    k = key   @ Wk.T + bk                    [B,N,M,C] -> [B,N,M,H,hd]
    attn = softmax_M(SCALE * einsum('bnhc,bnmhc->bnmh', q, k))
    out  = einsum('bnmh,bnmhc->bnhc', attn, k*k) . reshape(B,N,C)
    out  = out @ Wo.T + bo

Strategy: data parallel over the 16384 tokens (8 cores x 2048), no
collectives.  Projections run token-major: the activation tile (channel-major
in SBUF, 128 channels x 128 tokens) is the *stationary* matmul operand and
the weight chunk is the moving operand, so PSUM holds [token, channel_out]
tiles.  The whole attention middle (logits = per-head dot products, softmax
over the M=4 window, weighting of k^2) then runs on contiguous free-axis
DVE ops -- no indicator matmuls, no partition broadcasts, no strided
reductions.  Only the output projection needs channels back on partitions,
which costs 8 PE transposes of the 128x1024 attention output per token tile.
Per 128-token tile the PE does 96 GEMM matmuls (contraction 1024, free 512)
plus 8 transposes; DVE/ScalarE/DMA work is fully hidden underneath.
Host pre-transposes/casts inputs and weights to fp16 (error vs f32 reference
~1e-3, PE runs 16-bit at full rate), accumulation stays f32 in PSUM.
"""

import dataclasses
import numpy as np
from contextlib import ExitStack

try:
    import concourse.bass as bass
except ImportError:  # path fallback for bare containers
    import sys

    sys.path.insert(0, "/opt/trn_rl_repo")
    import concourse.bass as bass

import concourse.tile as tile
from concourse import bacc, mybir
from concourse.bass_utils import run_bass_kernel_spmd

# problem constants (hardcoded per the task contract)
B, N, C, H, HD, M = 4, 4096, 1024, 16, 64, 4
SCALE = float(HD) ** -0.5
NCORES = 8
T_TOTAL = B * N
T_CORE = T_TOTAL // NCORES  # 2048
TILE = 128                  # tokens per compute tile (PSUM partition limit)
TB = 512                    # tokens per DMA block
NJ = C // 128               # 8 channel chunks
NT = TB // TILE             # 4 tiles per block

DT = mybir.dt.bfloat16
import ml_dtypes
NPDT = ml_dtypes.bfloat16
F32 = mybir.dt.float32

EXP = mybir.ActivationFunctionType.Exp
AXX = mybir.AxisListType.X


def _bcast(ap, reps, axis):
    """Insert a 0-stride dim of size `reps` at AP position `axis` (0=partition)."""
    new = list(ap.ap)
    new.insert(axis, [0, reps])
    return dataclasses.replace(ap, ap=new)


def build_nc(t_core=T_CORE, with_bias=False):
    ntile = t_core // TILE
    nblk = t_core // TB
    nc = bacc.Bacc("TRN2", target_bir_lowering=False, debug=False)

    qT = nc.declare_dram_parameter("qT", [C, t_core], DT, isOutput=False)
    kT = nc.declare_dram_parameter("kT", [M, C, t_core], DT, isOutput=False)
    wqT = nc.declare_dram_parameter("wqT", [C, C], DT, isOutput=False)
    wkT = nc.declare_dram_parameter("wkT", [C, C], DT, isOutput=False)
    woT = nc.declare_dram_parameter("woT", [C, C], DT, isOutput=False)
    if with_bias:
        bq = nc.declare_dram_parameter("bq", [1, C], DT, isOutput=False)
        bk = nc.declare_dram_parameter("bk", [1, C], DT, isOutput=False)
        bo = nc.declare_dram_parameter("bo", [1, C], DT, isOutput=False)
    out = nc.declare_dram_parameter("out", [t_core, C], F32, isOutput=True)

    # DRAM views: channel dim split into (chunk, partition)
    qT_v = qT.ap().rearrange("(c p) t -> p c t", p=128)
    kT_v = kT.ap().rearrange("m (c p) t -> p m c t", p=128)
    wq_v = wqT.ap().rearrange("(c p) j -> p c j", p=128)
    wk_v = wkT.ap().rearrange("(c p) j -> p c j", p=128)
    wo_v = woT.ap().rearrange("(c p) j -> p c j", p=128)

    with tile.TileContext(nc) as tc, ExitStack() as ctx:
        consts = ctx.enter_context(tc.tile_pool(name="consts", bufs=1))
        p_inq = ctx.enter_context(tc.tile_pool(name="inq", bufs=2))
        p_ink = ctx.enter_context(tc.tile_pool(name="ink", bufs=8))
        p_qsb = ctx.enter_context(tc.tile_pool(name="qsb", bufs=2))
        p_ksb = ctx.enter_context(tc.tile_pool(name="ksb", bufs=2))
        p_prod = ctx.enter_context(tc.tile_pool(name="prod", bufs=1))
        p_ksq = ctx.enter_context(tc.tile_pool(name="ksq", bufs=1))
        p_sm = ctx.enter_context(tc.tile_pool(name="sm", bufs=2))
        p_ct = ctx.enter_context(tc.tile_pool(name="ct", bufs=1))
        p_y = ctx.enter_context(tc.tile_pool(name="y", bufs=2))
        p_ycm = ctx.enter_context(tc.tile_pool(name="ycm", bufs=2))
        p_osb = ctx.enter_context(tc.tile_pool(name="osb", bufs=2))
        pq = ctx.enter_context(tc.tile_pool(name="pq", bufs=1, space="PSUM"))
        pk = ctx.enter_context(tc.tile_pool(name="pk", bufs=2, space="PSUM"))
        po = ctx.enter_context(tc.tile_pool(name="po", bufs=1, space="PSUM"))

        # ---- constants / weights (resident, chunked so MMs depend on one DMA) ----
        wq_js = [consts.tile([128, C], DT, tag=f"wq{j}", name=f"wq{j}")
                 for j in range(NJ)]
        wk_js = [consts.tile([128, C], DT, tag=f"wk{j}", name=f"wk{j}")
                 for j in range(NJ)]
        wo_js = [consts.tile([128, C], DT, tag=f"wo{j}", name=f"wo{j}")
                 for j in range(NJ)]
        for j in range(NJ):
            nc.sync.dma_start(out=wq_js[j], in_=wq_v[:, j, :])
            nc.sync.dma_start(out=wk_js[j], in_=wk_v[:, j, :])
            nc.sync.dma_start(out=wo_js[j], in_=wo_v[:, j, :])
        if with_bias:
            ones_sb = consts.tile([1, TILE], DT)
            nc.vector.memset(ones_sb, 1.0)
            bq_sb = consts.tile([1, C], DT)
            bk_sb = consts.tile([1, C], DT)
            bo_sb = consts.tile([1, C], DT)
            nc.sync.dma_start(out=bq_sb, in_=bq.ap())
            nc.sync.dma_start(out=bk_sb, in_=bk.ap())
            nc.sync.dma_start(out=bo_sb, in_=bo.ap())

        def dma_block(blk):
            bsl = slice(blk * TB, (blk + 1) * TB)
            q_in = p_inq.tile([128, NJ, TB], DT, tag="qin", name="qin")
            nc.sync.dma_start(out=q_in, in_=qT_v[:, :, bsl])
            k_in = []
            for m in range(M):
                kt = p_ink.tile([128, NJ, TB], DT, tag="kin", name="kin")
                nc.sync.dma_start(out=kt, in_=kT_v[:, m, :, bsl])
                k_in.append(kt)
            return q_in, k_in

        cur = dma_block(0)
        nxt = None
        prevs = []  # queue of (y, t0) tiles awaiting transpose + O-proj (lag 2)

        for t in range(ntile + 2):
            tail = None
            if len(prevs) == 2 or (t >= ntile and prevs):
                tail = prevs.pop(0)
            if t < ntile:
                blk, tt = divmod(t, NT)
                if tt == 0 and t > 0:
                    cur, nxt = nxt, None
                q_in, k_in = cur
                tsl = slice(tt * TILE, (tt + 1) * TILE)

                # ---- Q projection (token-major: activations stationary) ----
                qp = pq.tile([128, 2, 512], F32, tag="pq", name="qp")
                for j in range(NJ):
                    for hf in range(2):
                        nc.tensor.matmul(
                            qp[:, hf, :],
                            q_in[:, j, tsl],
                            wq_js[j][:, hf * 512:(hf + 1) * 512],
                            start=(j == 0),
                            stop=(j == NJ - 1 and not with_bias),
                        )
                if with_bias:
                    for hf in range(2):
                        nc.tensor.matmul(
                            qp[:, hf, :], ones_sb,
                            bq_sb[:, hf * 512:(hf + 1) * 512],
                            start=False, stop=True,
                        )
                q_sb = p_qsb.tile([128, C], DT, tag="qsb", name="qsb")
                nc.scalar.copy(
                    out=q_sb.rearrange("p (u v) -> p u v", u=2), in_=qp
                )

            # ---- transpose of an earlier tile's attention output (DMA xbar) ----
            if tail is not None:
                y_pv, t0_pv = tail
                ycm = p_ycm.tile([128, NJ, TILE], DT, tag="ycm", name="ycm")
                for j in range(NJ):
                    nc.sync.dma_start_transpose(
                        ycm[:, j, :], y_pv[:, j * 128:(j + 1) * 128]
                    )

            if t < ntile:
                # prefetch next DMA block mid-way through this one
                if tt == 2 and blk + 1 < nblk:
                    nxt = dma_block(blk + 1)

                # ---- K projection ----
                k_sb = p_ksb.tile([128, M, C], DT, tag="ksb", name="ksb")
                for m in range(M):
                    kp = pk.tile([128, 2, 512], F32, tag="pk", name="kp")
                    for j in range(NJ):
                        for hf in range(2):
                            nc.tensor.matmul(
                                kp[:, hf, :],
                                k_in[m][:, j, tsl],
                                wk_js[j][:, hf * 512:(hf + 1) * 512],
                                start=(j == 0),
                                stop=(j == NJ - 1 and not with_bias),
                            )
                    if with_bias:
                        for hf in range(2):
                            nc.tensor.matmul(
                                kp[:, hf, :], ones_sb,
                                bk_sb[:, hf * 512:(hf + 1) * 512],
                                start=False, stop=True,
                            )
                    nc.scalar.copy(
                        out=k_sb[:, m, :].rearrange("p (u v) -> p u v", u=2),
                        in_=kp,
                    )

            # ---- output projection of the tailed tile ----
            if tail is not None:
                op = po.tile([128, 2, 512], F32, tag="po", name="op")
                for j in range(NJ):
                    for hf in range(2):
                        nc.tensor.matmul(
                            op[:, hf, :],
                            ycm[:, j, :],
                            wo_js[j][:, hf * 512:(hf + 1) * 512],
                            start=(j == 0),
                            stop=(j == NJ - 1 and not with_bias),
                        )
                if with_bias:
                    for hf in range(2):
                        nc.tensor.matmul(
                            op[:, hf, :], ones_sb,
                            bo_sb[:, hf * 512:(hf + 1) * 512],
                            start=False, stop=True,
                        )
                osb = p_osb.tile([128, C], F32, tag="osb", name="osb")
                nc.scalar.copy(
                    out=osb.rearrange("p (u v) -> p u v", u=2), in_=op
                )
                nc.sync.dma_start(
                    out=out.ap()[t0_pv:t0_pv + TILE, :], in_=osb
                )

            if t < ntile:
                # ---- attention middle, all free-axis ops (DVE + one exp) ----
                prod = p_prod.tile([128, M, C], DT, tag="prod", name="prod")
                nc.vector.tensor_mul(prod, _bcast(q_sb, M, 1), k_sb)
                ksq = p_ksq.tile([128, M, C], DT, tag="ksq", name="ksq")
                nc.vector.tensor_mul(ksq, k_sb, k_sb)
                lt = p_sm.tile([128, H, M], F32, tag="lt", name="lt")
                nc.vector.reduce_sum(
                    lt.rearrange("p h m -> p m h"),
                    prod.rearrange("p m (h x) -> p m h x", h=H),
                    axis=AXX,
                )
                e = p_sm.tile([128, H, M], F32, tag="e", name="e")
                nc.scalar.activation(e, lt, func=EXP, scale=SCALE)
                s = p_sm.tile([128, H], F32, tag="s", name="s")
                nc.vector.reduce_sum(s, e, axis=AXX)
                rcp = p_sm.tile([128, H], F32, tag="rcp", name="rcp")
                nc.vector.reciprocal(rcp, s)
                w = p_sm.tile([128, H, M], DT, tag="w", name="w")
                nc.vector.tensor_mul(w, e, _bcast(rcp, M, 2))
                ct = p_ct.tile([128, M, C], DT, tag="ct", name="ct")
                nc.vector.tensor_mul(
                    ct.rearrange("p m (h x) -> p m h x", h=H),
                    ksq.rearrange("p m (h x) -> p m h x", h=H),
                    _bcast(w.rearrange("p h m -> p m h"), HD, 3),
                )
                y01 = p_y.tile([128, C], DT, tag="y01", name="y01")
                nc.vector.tensor_add(y01, ct[:, 0, :], ct[:, 1, :])
                y23 = p_y.tile([128, C], DT, tag="y23", name="y23")
                nc.vector.tensor_add(y23, ct[:, 2, :], ct[:, 3, :])
                y = p_y.tile([128, C], DT, tag="y", name="y", bufs=3)
                nc.vector.tensor_add(y, y01, y23)
                prevs.append((y, t * TILE))

    nc.compile()
    return nc


def _host_prep(query, key, Wq, Wk, Wo, bq, bk, bo):
    qT = np.ascontiguousarray(query.reshape(T_TOTAL, C).T).astype(NPDT)
    kT = np.ascontiguousarray(key.reshape(T_TOTAL, M, C).transpose(1, 2, 0)).astype(NPDT)

    wqT = np.ascontiguousarray(Wq.T).astype(NPDT)
    wkT = np.ascontiguousarray(Wk.T).astype(NPDT)
    woT = np.ascontiguousarray(Wo.T).astype(NPDT)

    with_bias = bool(np.any(bq) or np.any(bk) or np.any(bo))
    common = {"wqT": wqT, "wkT": wkT, "woT": woT}
    if with_bias:
        common |= {
            "bq": bq.reshape(1, C).astype(NPDT),
            "bk": bk.reshape(1, C).astype(NPDT),
            "bo": bo.reshape(1, C).astype(NPDT),
        }
    in_maps = []
    for i in range(NCORES):
        sl = slice(i * T_CORE, (i + 1) * T_CORE)
        in_maps.append(
            {
                "qT": np.ascontiguousarray(qT[:, sl]),
                "kT": np.ascontiguousarray(kT[:, :, sl]),
                **common,
            }
        )
    return in_maps, with_bias


_NC_CACHE = {}
_LAST_RESULT = None


def kernel(query, key, gate, Wq, bq, Wk, bk, Wv, bv, Wg, bg, Wo, bo):
    in_maps, with_bias = _host_prep(query, key, Wq, Wk, Wo, bq, bk, bo)
    key_ = (T_CORE, with_bias)
    if key_ not in _NC_CACHE:
        _NC_CACHE[key_] = build_nc(T_CORE, with_bias)
    nc = _NC_CACHE[key_]
    res = run_bass_kernel_spmd(nc, in_maps, list(range(NCORES)))
    global _LAST_RESULT
    _LAST_RESULT = res
    out = np.concatenate([res.results[i]["out"] for i in range(NCORES)], axis=0)
    return out.reshape(B, N, C)


# revision 14
# speedup vs baseline: 1.7197x; 1.0340x over previous
"""Trainium2 Bass kernel for GatedCrossAttention (B=4, N=4096, C=1024, H=16, M=4).

Reference math (dead code removed: the v/gate projections are overwritten
by views of k in the original module, so v = g = k):
    q = query @ Wq.T + bq                    [B,N,C]   -> [B,N,H,hd]
    k = key   @ Wk.T + bk                    [B,N,M,C] -> [B,N,M,H,hd]
    attn = softmax_M(SCALE * einsum('bnhc,bnmhc->bnmh', q, k))
    out  = einsum('bnmh,bnmhc->bnhc', attn, k*k) . reshape(B,N,C)
    out  = out @ Wo.T + bo

Strategy: data parallel over the 16384 tokens (8 cores x 2048), no
collectives.  Projections run token-major: the activation tile (channel-major
in SBUF, 128 channels x 128 tokens) is the *stationary* matmul operand and
the weight chunk is the moving operand, so PSUM holds [token, channel_out]
tiles.  The whole attention middle (logits = per-head dot products, softmax
over the M=4 window, weighting of k^2) then runs on contiguous free-axis
DVE ops -- no indicator matmuls, no partition broadcasts, no strided
reductions.  Only the output projection needs channels back on partitions,
which costs 8 PE transposes of the 128x1024 attention output per token tile.
Per 128-token tile the PE does 96 GEMM matmuls (contraction 1024, free 512)
plus 8 transposes; DVE/ScalarE/DMA work is fully hidden underneath.
Host pre-transposes/casts inputs and weights to fp16 (error vs f32 reference
~1e-3, PE runs 16-bit at full rate), accumulation stays f32 in PSUM.
"""

import dataclasses
import numpy as np
from contextlib import ExitStack

try:
    import concourse.bass as bass
except ImportError:  # path fallback for bare containers
    import sys

    sys.path.insert(0, "/opt/trn_rl_repo")
    import concourse.bass as bass

import concourse.tile as tile
from concourse import bacc, mybir
from concourse.bass_utils import run_bass_kernel_spmd

# problem constants (hardcoded per the task contract)
B, N, C, H, HD, M = 4, 4096, 1024, 16, 64, 4
SCALE = float(HD) ** -0.5
NCORES = 8
T_TOTAL = B * N
T_CORE = T_TOTAL // NCORES  # 2048
TILE = 128                  # tokens per compute tile (PSUM partition limit)
TB = 512                    # tokens per DMA block
NJ = C // 128               # 8 channel chunks
NT = TB // TILE             # 4 tiles per block

DT = mybir.dt.bfloat16
import ml_dtypes
NPDT = ml_dtypes.bfloat16
F32 = mybir.dt.float32

EXP = mybir.ActivationFunctionType.Exp
AXX = mybir.AxisListType.X


def _bcast(ap, reps, axis):
    """Insert a 0-stride dim of size `reps` at AP position `axis` (0=partition)."""
    new = list(ap.ap)
    new.insert(axis, [0, reps])
    return dataclasses.replace(ap, ap=new)


def build_nc(t_core=T_CORE, with_bias=False):
    ntile = t_core // TILE
    nblk = t_core // TB
    nc = bacc.Bacc("TRN2", target_bir_lowering=False, debug=False)

    # inputs host-tiled so each DMA reads one contiguous 8KB run per partition:
    # qT[b, p, j, t] = query_ct[j*128+p, b*TB+t]
    qTd = nc.declare_dram_parameter("qT", [t_core // TB, 128, NJ * TB], DT,
                                    isOutput=False)
    kTd = nc.declare_dram_parameter("kT", [M, t_core // TB, 128, NJ * TB], DT,
                                    isOutput=False)
    wqT = nc.declare_dram_parameter("wqT", [C, C], DT, isOutput=False)
    wkT = nc.declare_dram_parameter("wkT", [C, C], DT, isOutput=False)
    woT = nc.declare_dram_parameter("woT", [C, C], DT, isOutput=False)
    if with_bias:
        bq = nc.declare_dram_parameter("bq", [1, C], DT, isOutput=False)
        bk = nc.declare_dram_parameter("bk", [1, C], DT, isOutput=False)
        bo = nc.declare_dram_parameter("bo", [1, C], DT, isOutput=False)
    out = nc.declare_dram_parameter("out", [t_core, C], F32, isOutput=True)

    # DRAM views
    qT_v = qTd.ap().rearrange("b p (j t) -> b p j t", j=NJ)
    kT_v = kTd.ap().rearrange("m b p (j t) -> m b p j t", j=NJ)
    wq_v = wqT.ap().rearrange("(c p) j -> p c j", p=128)
    wk_v = wkT.ap().rearrange("(c p) j -> p c j", p=128)
    # xbar transpose writes ycm[p, j, t] = y[t, j*128+p]: standard chunking
    wo_v = woT.ap().rearrange("(c p) j -> p c j", p=128)

    with tile.TileContext(nc) as tc, ExitStack() as ctx:
        consts = ctx.enter_context(tc.tile_pool(name="consts", bufs=1))
        p_inq = ctx.enter_context(tc.tile_pool(name="inq", bufs=2))
        p_ink = ctx.enter_context(tc.tile_pool(name="ink", bufs=8))
        p_qsb = ctx.enter_context(tc.tile_pool(name="qsb", bufs=2))
        p_ksb = ctx.enter_context(tc.tile_pool(name="ksb", bufs=2))
        p_prod = ctx.enter_context(tc.tile_pool(name="prod", bufs=1))
        p_ksq = ctx.enter_context(tc.tile_pool(name="ksq", bufs=1))
        p_sm = ctx.enter_context(tc.tile_pool(name="sm", bufs=2))
        p_ct = ctx.enter_context(tc.tile_pool(name="ct", bufs=1))
        p_y = ctx.enter_context(tc.tile_pool(name="y", bufs=2))
        p_ycm = ctx.enter_context(tc.tile_pool(name="ycm", bufs=2))
        p_osb = ctx.enter_context(tc.tile_pool(name="osb", bufs=2))
        pq = ctx.enter_context(tc.tile_pool(name="pq", bufs=1, space="PSUM"))
        pk = ctx.enter_context(tc.tile_pool(name="pk", bufs=2, space="PSUM"))
        po = ctx.enter_context(tc.tile_pool(name="po", bufs=1, space="PSUM"))

        # ---- constants / weights (resident, chunked so MMs depend on one DMA) ----
        wq_js = [consts.tile([128, C], DT, tag=f"wq{j}", name=f"wq{j}")
                 for j in range(NJ)]
        wk_js = [consts.tile([128, C], DT, tag=f"wk{j}", name=f"wk{j}")
                 for j in range(NJ)]
        wo_js = [consts.tile([128, C], DT, tag=f"wo{j}", name=f"wo{j}")
                 for j in range(NJ)]
        for j in range(NJ):
            nc.sync.dma_start(out=wq_js[j], in_=wq_v[:, j, :])
        if with_bias:
            ones_sb = consts.tile([1, TILE], DT)
            nc.vector.memset(ones_sb, 1.0)
            bq_sb = consts.tile([1, C], DT)
            bk_sb = consts.tile([1, C], DT)
            bo_sb = consts.tile([1, C], DT)
            nc.sync.dma_start(out=bq_sb, in_=bq.ap())
            nc.sync.dma_start(out=bk_sb, in_=bk.ap())
            nc.sync.dma_start(out=bo_sb, in_=bo.ap())

        def dma_q(blk):
            q_in = p_inq.tile([128, NJ, TB], DT, tag="qin", name="qin")
            nc.sync.dma_start(out=q_in, in_=qT_v[blk])
            return q_in

        def dma_k(blk):
            k_in = []
            for m in range(M):
                kt = p_ink.tile([128, NJ, TB], DT, tag="kin", name="kin")
                nc.sync.dma_start(out=kt, in_=kT_v[m, blk])
                k_in.append(kt)
            return k_in

        def dma_block(blk):
            return dma_q(blk), dma_k(blk)

        # startup order: wq (above) + q block 0 gate the first matmul; then
        # wk + k block 0; wo arrives while block 0 computes.
        q0 = dma_q(0)
        for j in range(NJ):
            nc.sync.dma_start(out=wk_js[j], in_=wk_v[:, j, :])
        k0 = dma_k(0)
        for j in range(NJ):
            nc.sync.dma_start(out=wo_js[j], in_=wo_v[:, j, :])
        cur = (q0, k0)
        nxt = None

        # PE warmup: dummy matmuls during the initial DMA fill keep the HAM
        # activity window busy so real matmuls start at full clock.
        wdum = consts.tile([128, 512], DT)
        nc.vector.memset(wdum, 0.0)
        wps = po.tile([128, 2, 512], F32, tag="po", name="wps")
        for i in range(22):
            nc.tensor.matmul(wps[:, i % 2, :], wdum[:, :128], wdum,
                             start=True, stop=True)
        prevs = []  # queue of (y, t0) tiles awaiting transpose + O-proj (lag 2)

        for t in range(ntile + 2):
            tail = None
            if len(prevs) == 2 or (t >= ntile and prevs):
                tail = prevs.pop(0)
            if t < ntile:
                blk, tt = divmod(t, NT)
                if tt == 0 and t > 0:
                    cur, nxt = nxt, None
                q_in, k_in = cur
                tsl = slice(tt * TILE, (tt + 1) * TILE)

                # ---- Q projection (token-major: activations stationary) ----
                qp = pq.tile([128, 2, 512], F32, tag="pq", name="qp")
                for j in range(NJ):
                    for hf in range(2):
                        nc.tensor.matmul(
                            qp[:, hf, :],
                            q_in[:, j, tsl],
                            wq_js[j][:, hf * 512:(hf + 1) * 512],
                            start=(j == 0),
                            stop=(j == NJ - 1 and not with_bias),
                        )
                if with_bias:
                    for hf in range(2):
                        nc.tensor.matmul(
                            qp[:, hf, :], ones_sb,
                            bq_sb[:, hf * 512:(hf + 1) * 512],
                            start=False, stop=True,
                        )
                q_sb = p_qsb.tile([128, C], DT, tag="qsb", name="qsb")
                nc.scalar.copy(
                    out=q_sb.rearrange("p (u v) -> p u v", u=2), in_=qp
                )

            # ---- transpose of an earlier tile's attention output (DMA xbar) ----
            if tail is not None:
                y_pv, t0_pv = tail
                ycm = p_ycm.tile([128, NJ, TILE], DT, tag="ycm", name="ycm")
                nc.sync.dma_start_transpose(ycm, y_pv)

            if t < ntile:
                # prefetch next DMA block mid-way through this one
                if tt == 2 and blk + 1 < nblk:
                    nxt = dma_block(blk + 1)

                # ---- K projection ----
                k_sb = p_ksb.tile([128, M, C], DT, tag="ksb", name="ksb")
                for m in range(M):
                    kp = pk.tile([128, 2, 512], F32, tag="pk", name="kp")
                    for j in range(NJ):
                        for hf in range(2):
                            nc.tensor.matmul(
                                kp[:, hf, :],
                                k_in[m][:, j, tsl],
                                wk_js[j][:, hf * 512:(hf + 1) * 512],
                                start=(j == 0),
                                stop=(j == NJ - 1 and not with_bias),
                            )
                    if with_bias:
                        for hf in range(2):
                            nc.tensor.matmul(
                                kp[:, hf, :], ones_sb,
                                bk_sb[:, hf * 512:(hf + 1) * 512],
                                start=False, stop=True,
                            )
                    nc.scalar.copy(
                        out=k_sb[:, m, :].rearrange("p (u v) -> p u v", u=2),
                        in_=kp,
                    )

            # ---- output projection of the tailed tile ----
            if tail is not None:
                op = po.tile([128, 2, 512], F32, tag="po", name="op")
                for j in range(NJ):
                    for hf in range(2):
                        nc.tensor.matmul(
                            op[:, hf, :],
                            ycm[:, j, :],
                            wo_js[j][:, hf * 512:(hf + 1) * 512],
                            start=(j == 0),
                            stop=(j == NJ - 1 and not with_bias),
                        )
                if with_bias:
                    for hf in range(2):
                        nc.tensor.matmul(
                            op[:, hf, :], ones_sb,
                            bo_sb[:, hf * 512:(hf + 1) * 512],
                            start=False, stop=True,
                        )
                osb = p_osb.tile([128, C], F32, tag="osb", name="osb")
                nc.scalar.copy(
                    out=osb.rearrange("p (u v) -> p u v", u=2), in_=op
                )
                nc.sync.dma_start(
                    out=out.ap()[t0_pv:t0_pv + TILE, :], in_=osb
                )

            if t < ntile:
                # ---- attention middle, all free-axis ops (DVE + one exp) ----
                prod = p_prod.tile([128, M, C], DT, tag="prod", name="prod")
                nc.vector.tensor_mul(prod, _bcast(q_sb, M, 1), k_sb)
                ksq = p_ksq.tile([128, M, C], DT, tag="ksq", name="ksq")
                nc.vector.tensor_mul(ksq, k_sb, k_sb)
                lt = p_sm.tile([128, H, M], F32, tag="lt", name="lt")
                nc.vector.reduce_sum(
                    lt.rearrange("p h m -> p m h"),
                    prod.rearrange("p m (h x) -> p m h x", h=H),
                    axis=AXX,
                )
                e = p_sm.tile([128, H, M], F32, tag="e", name="e")
                nc.scalar.activation(e, lt, func=EXP, scale=SCALE)
                s = p_sm.tile([128, H], F32, tag="s", name="s")
                nc.vector.reduce_sum(s, e, axis=AXX)
                rcp = p_sm.tile([128, H], F32, tag="rcp", name="rcp")
                nc.vector.reciprocal(rcp, s)
                w = p_sm.tile([128, H, M], DT, tag="w", name="w")
                nc.vector.tensor_mul(w, e, _bcast(rcp, M, 2))
                ct = p_ct.tile([128, M, C], DT, tag="ct", name="ct")
                nc.vector.tensor_mul(
                    ct.rearrange("p m (h x) -> p m h x", h=H),
                    ksq.rearrange("p m (h x) -> p m h x", h=H),
                    _bcast(w.rearrange("p h m -> p m h"), HD, 3),
                )
                y01 = p_y.tile([128, C], DT, tag="y01", name="y01")
                nc.vector.tensor_add(y01, ct[:, 0, :], ct[:, 1, :])
                y23 = p_y.tile([128, C], DT, tag="y23", name="y23")
                nc.vector.tensor_add(y23, ct[:, 2, :], ct[:, 3, :])
                y = p_y.tile([128, C], DT, tag="y", name="y", bufs=3)
                nc.vector.tensor_add(y, y01, y23)
                prevs.append((y, t * TILE))

    nc.compile()
    return nc


def _host_prep(query, key, Wq, Wk, Wo, bq, bk, bo):
    nblk = T_CORE // TB
    # qT[core][b, p, j*TB+t] = query_ct[j*128+p, core*T_CORE + b*TB+t]
    qT = (np.asarray(query, np.float32).reshape(T_TOTAL, C).T
          .reshape(NJ, 128, NCORES, nblk, TB)
          .transpose(2, 3, 1, 0, 4)
          .reshape(NCORES, nblk, 128, NJ * TB)).astype(NPDT)
    kT = (np.asarray(key, np.float32).reshape(T_TOTAL, M, C).transpose(1, 2, 0)
          .reshape(M, NJ, 128, NCORES, nblk, TB)
          .transpose(3, 0, 4, 2, 1, 5)
          .reshape(NCORES, M, nblk, 128, NJ * TB)).astype(NPDT)

    wqT = np.ascontiguousarray(Wq.T).astype(NPDT)
    wkT = np.ascontiguousarray(Wk.T).astype(NPDT)
    woT = np.ascontiguousarray(Wo.T).astype(NPDT)

    with_bias = bool(np.any(bq) or np.any(bk) or np.any(bo))
    common = {"wqT": wqT, "wkT": wkT, "woT": woT}
    if with_bias:
        common |= {
            "bq": bq.reshape(1, C).astype(NPDT),
            "bk": bk.reshape(1, C).astype(NPDT),
            "bo": bo.reshape(1, C).astype(NPDT),
        }
    in_maps = []
    for i in range(NCORES):
        in_maps.append(
            {
                "qT": np.ascontiguousarray(qT[i]),
                "kT": np.ascontiguousarray(kT[i]),
                **common,
            }
        )
    return in_maps, with_bias


_NC_CACHE = {}
_LAST_RESULT = None


def kernel(query, key, gate, Wq, bq, Wk, bk, Wv, bv, Wg, bg, Wo, bo):
    in_maps, with_bias = _host_prep(query, key, Wq, Wk, Wo, bq, bk, bo)
    key_ = (T_CORE, with_bias)
    if key_ not in _NC_CACHE:
        _NC_CACHE[key_] = build_nc(T_CORE, with_bias)
    nc = _NC_CACHE[key_]
    res = run_bass_kernel_spmd(nc, in_maps, list(range(NCORES)))
    global _LAST_RESULT
    _LAST_RESULT = res
    out = np.concatenate([res.results[i]["out"] for i in range(NCORES)], axis=0)
    return out.reshape(B, N, C)


# revision 15
# speedup vs baseline: 1.7235x; 1.0022x over previous
"""Trainium2 Bass kernel for GatedCrossAttention (B=4, N=4096, C=1024, H=16, M=4).

Reference math (dead code removed: the v/gate projections are overwritten
by views of k in the original module, so v = g = k):
    q = query @ Wq.T + bq                    [B,N,C]   -> [B,N,H,hd]
    k = key   @ Wk.T + bk                    [B,N,M,C] -> [B,N,M,H,hd]
    attn = softmax_M(SCALE * einsum('bnhc,bnmhc->bnmh', q, k))
    out  = einsum('bnmh,bnmhc->bnhc', attn, k*k) . reshape(B,N,C)
    out  = out @ Wo.T + bo

Strategy: data parallel over the 16384 tokens (8 cores x 2048), no
collectives.  Projections run token-major: the activation tile (channel-major
in SBUF, 128 channels x 128 tokens) is the *stationary* matmul operand and
the weight chunk is the moving operand, so PSUM holds [token, channel_out]
tiles.  The whole attention middle (logits = per-head dot products, softmax
over the M=4 window, weighting of k^2) then runs on contiguous free-axis
DVE ops -- no indicator matmuls, no partition broadcasts, no strided
reductions.  Only the output projection needs channels back on partitions,
which costs 8 PE transposes of the 128x1024 attention output per token tile.
Per 128-token tile the PE does 96 GEMM matmuls (contraction 1024, free 512)
plus 8 transposes; DVE/ScalarE/DMA work is fully hidden underneath.
Host pre-transposes/casts inputs and weights to fp16 (error vs f32 reference
~1e-3, PE runs 16-bit at full rate), accumulation stays f32 in PSUM.
"""

import dataclasses
import numpy as np
from contextlib import ExitStack

try:
    import concourse.bass as bass
except ImportError:  # path fallback for bare containers
    import sys

    sys.path.insert(0, "/opt/trn_rl_repo")
    import concourse.bass as bass

import concourse.tile as tile
from concourse import bacc, mybir
from concourse.bass_utils import run_bass_kernel_spmd

# problem constants (hardcoded per the task contract)
B, N, C, H, HD, M = 4, 4096, 1024, 16, 64, 4
SCALE = float(HD) ** -0.5
NCORES = 8
T_TOTAL = B * N
T_CORE = T_TOTAL // NCORES  # 2048
TILE = 128                  # tokens per compute tile (PSUM partition limit)
TB = 512                    # tokens per DMA block
NJ = C // 128               # 8 channel chunks
NT = TB // TILE             # 4 tiles per block

DT = mybir.dt.bfloat16
import ml_dtypes
NPDT = ml_dtypes.bfloat16
F32 = mybir.dt.float32

EXP = mybir.ActivationFunctionType.Exp
AXX = mybir.AxisListType.X


def _bcast(ap, reps, axis):
    """Insert a 0-stride dim of size `reps` at AP position `axis` (0=partition)."""
    new = list(ap.ap)
    new.insert(axis, [0, reps])
    return dataclasses.replace(ap, ap=new)


def build_nc(t_core=T_CORE, with_bias=False):
    ntile = t_core // TILE
    nblk = t_core // TB
    nc = bacc.Bacc("TRN2", target_bir_lowering=False, debug=False)

    # inputs host-tiled so each DMA reads one contiguous 8KB run per partition:
    # qT[b, p, j, t] = query_ct[j*128+p, b*TB+t]
    qTd = nc.declare_dram_parameter("qT", [t_core // TB, 128, NJ * TB], DT,
                                    isOutput=False)
    kTd = nc.declare_dram_parameter("kT", [M, t_core // TB, 128, NJ * TB], DT,
                                    isOutput=False)
    wqT = nc.declare_dram_parameter("wqT", [C, C], DT, isOutput=False)
    wkT = nc.declare_dram_parameter("wkT", [C, C], DT, isOutput=False)
    woT = nc.declare_dram_parameter("woT", [C, C], DT, isOutput=False)
    if with_bias:
        bq = nc.declare_dram_parameter("bq", [1, C], DT, isOutput=False)
        bk = nc.declare_dram_parameter("bk", [1, C], DT, isOutput=False)
        bo = nc.declare_dram_parameter("bo", [1, C], DT, isOutput=False)
    out = nc.declare_dram_parameter("out", [t_core, C], F32, isOutput=True)

    # DRAM views
    qT_v = qTd.ap().rearrange("b p (j t) -> b p j t", j=NJ)
    kT_v = kTd.ap().rearrange("m b p (j t) -> m b p j t", j=NJ)
    wq_v = wqT.ap().rearrange("(c p) j -> p c j", p=128)
    wk_v = wkT.ap().rearrange("(c p) j -> p c j", p=128)
    # xbar transpose writes ycm[p, j, t] = y[t, j*128+p]: standard chunking
    wo_v = woT.ap().rearrange("(c p) j -> p c j", p=128)

    with tile.TileContext(nc) as tc, ExitStack() as ctx:
        consts = ctx.enter_context(tc.tile_pool(name="consts", bufs=1))
        p_inq = ctx.enter_context(tc.tile_pool(name="inq", bufs=2))
        p_ink = ctx.enter_context(tc.tile_pool(name="ink", bufs=8))
        p_qsb = ctx.enter_context(tc.tile_pool(name="qsb", bufs=2))
        p_ksb = ctx.enter_context(tc.tile_pool(name="ksb", bufs=2))
        p_prod = ctx.enter_context(tc.tile_pool(name="prod", bufs=1))
        p_ksq = ctx.enter_context(tc.tile_pool(name="ksq", bufs=1))
        p_sm = ctx.enter_context(tc.tile_pool(name="sm", bufs=2))
        p_ct = ctx.enter_context(tc.tile_pool(name="ct", bufs=1))
        p_y = ctx.enter_context(tc.tile_pool(name="y", bufs=2))
        p_ycm = ctx.enter_context(tc.tile_pool(name="ycm", bufs=2))
        p_osb = ctx.enter_context(tc.tile_pool(name="osb", bufs=2))
        pq = ctx.enter_context(tc.tile_pool(name="pq", bufs=1, space="PSUM"))
        pk = ctx.enter_context(tc.tile_pool(name="pk", bufs=2, space="PSUM"))
        po = ctx.enter_context(tc.tile_pool(name="po", bufs=1, space="PSUM"))

        # ---- constants / weights (resident, chunked so MMs depend on one DMA) ----
        wq_js = [consts.tile([128, C], DT, tag=f"wq{j}", name=f"wq{j}")
                 for j in range(NJ)]
        wk_js = [consts.tile([128, C], DT, tag=f"wk{j}", name=f"wk{j}")
                 for j in range(NJ)]
        wo_js = [consts.tile([128, C], DT, tag=f"wo{j}", name=f"wo{j}")
                 for j in range(NJ)]
        for j in range(NJ):
            nc.sync.dma_start(out=wq_js[j], in_=wq_v[:, j, :])
        if with_bias:
            ones_sb = consts.tile([1, TILE], DT)
            nc.vector.memset(ones_sb, 1.0)
            bq_sb = consts.tile([1, C], DT)
            bk_sb = consts.tile([1, C], DT)
            bo_sb = consts.tile([1, C], DT)
            nc.sync.dma_start(out=bq_sb, in_=bq.ap())
            nc.sync.dma_start(out=bk_sb, in_=bk.ap())
            nc.sync.dma_start(out=bo_sb, in_=bo.ap())

        def dma_q(blk):
            q_in = p_inq.tile([128, NJ, TB], DT, tag="qin", name="qin")
            for h in range(2):
                js = slice(h * NJ // 2, (h + 1) * NJ // 2)
                nc.sync.dma_start(out=q_in[:, js, :], in_=qT_v[blk][:, js, :])
            return q_in

        def dma_k(blk):
            k_in = []
            for m in range(M):
                kt = p_ink.tile([128, NJ, TB], DT, tag="kin", name="kin")
                for h in range(2):
                    js = slice(h * NJ // 2, (h + 1) * NJ // 2)
                    nc.sync.dma_start(out=kt[:, js, :], in_=kT_v[m, blk][:, js, :])
                k_in.append(kt)
            return k_in

        def dma_block(blk):
            return dma_q(blk), dma_k(blk)

        # startup order: wq (above) + q block 0 gate the first matmul; then
        # wk + k block 0; wo arrives while block 0 computes.
        q0 = dma_q(0)  # noqa: startup priority
        for j in range(NJ):
            nc.sync.dma_start(out=wk_js[j], in_=wk_v[:, j, :])
        k0 = dma_k(0)
        for j in range(NJ):
            nc.sync.dma_start(out=wo_js[j], in_=wo_v[:, j, :])
        cur = (q0, k0)
        nxt = None

        # PE warmup: dummy matmuls during the initial DMA fill keep the HAM
        # activity window busy so real matmuls start at full clock.
        wdum = consts.tile([128, 512], DT)
        nc.vector.memset(wdum, 0.0)
        wps = po.tile([128, 2, 512], F32, tag="po", name="wps")
        for i in range(14):
            nc.tensor.matmul(wps[:, i % 2, :], wdum[:, :128], wdum,
                             start=True, stop=True)
        prevs = []  # queue of (y, t0) tiles awaiting transpose + O-proj (lag 2)

        for t in range(ntile + 2):
            tail = None
            if len(prevs) == 2 or (t >= ntile and prevs):
                tail = prevs.pop(0)
            if t < ntile:
                blk, tt = divmod(t, NT)
                if tt == 0 and t > 0:
                    cur, nxt = nxt, None
                q_in, k_in = cur
                tsl = slice(tt * TILE, (tt + 1) * TILE)

                # ---- Q projection (token-major: activations stationary) ----
                qp = pq.tile([128, 2, 512], F32, tag="pq", name="qp")
                for j in range(NJ):
                    for hf in range(2):
                        nc.tensor.matmul(
                            qp[:, hf, :],
                            q_in[:, j, tsl],
                            wq_js[j][:, hf * 512:(hf + 1) * 512],
                            start=(j == 0),
                            stop=(j == NJ - 1 and not with_bias),
                        )
                if with_bias:
                    for hf in range(2):
                        nc.tensor.matmul(
                            qp[:, hf, :], ones_sb,
                            bq_sb[:, hf * 512:(hf + 1) * 512],
                            start=False, stop=True,
                        )
                q_sb = p_qsb.tile([128, C], DT, tag="qsb", name="qsb")
                nc.scalar.copy(
                    out=q_sb.rearrange("p (u v) -> p u v", u=2), in_=qp
                )

            # ---- transpose of an earlier tile's attention output (DMA xbar) ----
            if tail is not None:
                y_pv, t0_pv = tail
                ycm = p_ycm.tile([128, NJ, TILE], DT, tag="ycm", name="ycm")
                for s in range(4):
                    nc.sync.dma_start_transpose(
                        ycm[:, 2 * s:2 * s + 2, :],
                        y_pv[:, s * 256:(s + 1) * 256],
                    )

            if t < ntile:
                # prefetch next DMA block mid-way through this one
                if tt == 2 and blk + 1 < nblk:
                    nxt = dma_block(blk + 1)

                # ---- K projection ----
                k_sb = p_ksb.tile([128, M, C], DT, tag="ksb", name="ksb")
                for m in range(M):
                    kp = pk.tile([128, 2, 512], F32, tag="pk", name="kp")
                    for j in range(NJ):
                        for hf in range(2):
                            nc.tensor.matmul(
                                kp[:, hf, :],
                                k_in[m][:, j, tsl],
                                wk_js[j][:, hf * 512:(hf + 1) * 512],
                                start=(j == 0),
                                stop=(j == NJ - 1 and not with_bias),
                            )
                    if with_bias:
                        for hf in range(2):
                            nc.tensor.matmul(
                                kp[:, hf, :], ones_sb,
                                bk_sb[:, hf * 512:(hf + 1) * 512],
                                start=False, stop=True,
                            )
                    nc.scalar.copy(
                        out=k_sb[:, m, :].rearrange("p (u v) -> p u v", u=2),
                        in_=kp,
                    )

            # ---- output projection of the tailed tile ----
            if tail is not None:
                op = po.tile([128, 2, 512], F32, tag="po", name="op")
                for j in range(NJ):
                    for hf in range(2):
                        nc.tensor.matmul(
                            op[:, hf, :],
                            ycm[:, j, :],
                            wo_js[j][:, hf * 512:(hf + 1) * 512],
                            start=(j == 0),
                            stop=(j == NJ - 1 and not with_bias),
                        )
                if with_bias:
                    for hf in range(2):
                        nc.tensor.matmul(
                            op[:, hf, :], ones_sb,
                            bo_sb[:, hf * 512:(hf + 1) * 512],
                            start=False, stop=True,
                        )
                osb = p_osb.tile([128, C], F32, tag="osb", name="osb")
                nc.scalar.copy(
                    out=osb.rearrange("p (u v) -> p u v", u=2), in_=op
                )
                for h in range(2):
                    nc.sync.dma_start(
                        out=out.ap()[t0_pv:t0_pv + TILE, h * 512:(h + 1) * 512],
                        in_=osb[:, h * 512:(h + 1) * 512],
                    )

            if t < ntile:
                # ---- attention middle, all free-axis ops (DVE + one exp) ----
                prod = p_prod.tile([128, M, C], DT, tag="prod", name="prod")
                nc.vector.tensor_mul(prod, _bcast(q_sb, M, 1), k_sb)
                ksq = p_ksq.tile([128, M, C], DT, tag="ksq", name="ksq")
                nc.vector.tensor_mul(ksq, k_sb, k_sb)
                lt = p_sm.tile([128, H, M], F32, tag="lt", name="lt")
                nc.vector.reduce_sum(
                    lt.rearrange("p h m -> p m h"),
                    prod.rearrange("p m (h x) -> p m h x", h=H),
                    axis=AXX,
                )
                e = p_sm.tile([128, H, M], F32, tag="e", name="e")
                nc.scalar.activation(e, lt, func=EXP, scale=SCALE)
                s = p_sm.tile([128, H], F32, tag="s", name="s")
                nc.vector.reduce_sum(s, e, axis=AXX)
                rcp = p_sm.tile([128, H], F32, tag="rcp", name="rcp")
                nc.vector.reciprocal(rcp, s)
                w = p_sm.tile([128, H, M], DT, tag="w", name="w")
                nc.vector.tensor_mul(w, e, _bcast(rcp, M, 2))
                ct = p_ct.tile([128, M, C], DT, tag="ct", name="ct")
                nc.vector.tensor_mul(
                    ct.rearrange("p m (h x) -> p m h x", h=H),
                    ksq.rearrange("p m (h x) -> p m h x", h=H),
                    _bcast(w.rearrange("p h m -> p m h"), HD, 3),
                )
                y01 = p_y.tile([128, C], DT, tag="y01", name="y01")
                nc.vector.tensor_add(y01, ct[:, 0, :], ct[:, 1, :])
                y23 = p_y.tile([128, C], DT, tag="y23", name="y23")
                nc.vector.tensor_add(y23, ct[:, 2, :], ct[:, 3, :])
                y = p_y.tile([128, C], DT, tag="y", name="y", bufs=3)
                nc.vector.tensor_add(y, y01, y23)
                prevs.append((y, t * TILE))

    nc.compile()
    return nc


def _host_prep(query, key, Wq, Wk, Wo, bq, bk, bo):
    nblk = T_CORE // TB
    # qT[core][b, p, j*TB+t] = query_ct[j*128+p, core*T_CORE + b*TB+t]
    qT = (np.asarray(query, np.float32).reshape(T_TOTAL, C).T
          .reshape(NJ, 128, NCORES, nblk, TB)
          .transpose(2, 3, 1, 0, 4)
          .reshape(NCORES, nblk, 128, NJ * TB)).astype(NPDT)
    kT = (np.asarray(key, np.float32).reshape(T_TOTAL, M, C).transpose(1, 2, 0)
          .reshape(M, NJ, 128, NCORES, nblk, TB)
          .transpose(3, 0, 4, 2, 1, 5)
          .reshape(NCORES, M, nblk, 128, NJ * TB)).astype(NPDT)

    wqT = np.ascontiguousarray(Wq.T).astype(NPDT)
    wkT = np.ascontiguousarray(Wk.T).astype(NPDT)
    woT = np.ascontiguousarray(Wo.T).astype(NPDT)

    with_bias = bool(np.any(bq) or np.any(bk) or np.any(bo))
    common = {"wqT": wqT, "wkT": wkT, "woT": woT}
    if with_bias:
        common |= {
            "bq": bq.reshape(1, C).astype(NPDT),
            "bk": bk.reshape(1, C).astype(NPDT),
            "bo": bo.reshape(1, C).astype(NPDT),
        }
    in_maps = []
    for i in range(NCORES):
        in_maps.append(
            {
                "qT": np.ascontiguousarray(qT[i]),
                "kT": np.ascontiguousarray(kT[i]),
                **common,
            }
        )
    return in_maps, with_bias


_NC_CACHE = {}
_LAST_RESULT = None


def kernel(query, key, gate, Wq, bq, Wk, bk, Wv, bv, Wg, bg, Wo, bo):
    in_maps, with_bias = _host_prep(query, key, Wq, Wk, Wo, bq, bk, bo)
    key_ = (T_CORE, with_bias)
    if key_ not in _NC_CACHE:
        _NC_CACHE[key_] = build_nc(T_CORE, with_bias)
    nc = _NC_CACHE[key_]
    res = run_bass_kernel_spmd(nc, in_maps, list(range(NCORES)))
    global _LAST_RESULT
    _LAST_RESULT = res
    out = np.concatenate([res.results[i]["out"] for i in range(NCORES)], axis=0)
    return out.reshape(B, N, C)
